# revision 21
# baseline (speedup 1.0000x reference)
"""DSSIM loss kernel for Trainium2, 8 NeuronCores, data-parallel over batch.

Math: for each (b, c) 512x512 image pair (x, y):
  s = x + y, d = x - y                       (prep, bf16)
  S = conv(s), D = conv(d), P = conv(s^2), Q = conv(d^2)  (separable 11-tap)
  u = 2*mu1*mu2       = (S^2 - D^2)/2
  v = mu1^2 + mu2^2   = (S^2 + D^2)/2
  w - u = 2*sigma12 + C2       = (P - Q)/2 + C2 - u
  h - v = sigma1 + sigma2 + C2 = (P + Q)/2 + C2 - v
  ssim = (u + C1)(w - u) / ((v + C1)(h - v));  DSSIM = 1 - mean(ssim)

Structure per (b, c) set:
  prep computes s, d, r = x*y (vector/gpsimd) and 2x^2, 2y^2 (scalar
  Squares with scale sqrt2), q = 2x^2 + 2y^2 (gpsimd). Since
  s^2 - d^2 = 4xy and s^2 + d^2 = q, the quadratic convs land in the +/-
  basis directly: r's pass1 band is an exactly-4x-scaled copy of G.
  pass1 (image chunk as PE-stationary operand) convolves H and transposes;
  both PSUM pairs evacuate as plain bf16 casts (an engine op may read only
  ONE PSUM operand, and GPSIMD cannot touch PSUM at all). pass2 is four
  matmuls (a matmul output must fit one PSUM bank) sharing one lhsT:
    psA = (G/2) . [t1s, t1d]  = [S/2, D/2]
    psB = (G/2) . [t1r, t1q]  = [(P-Q)/2, (P+Q)/2]
  Map stage reads PSUM directly (one PSUM operand per op):
    ab = Square(psA * sqrt2) = [S^2/2, D^2/2]        (scalar)
    uv = [ab0 - ab1, ab0 + ab1] = [u, v]             (gpsimd)
    nd = (psB + C2) - uv = [w-u, h-v]                (vector STT)
    numden = (uv + C1) * nd                          (vector STT)
    ssim = numden0 * recip(numden1), accumulated per-partition into rsums.
Host reduces the per-core partition sums.

Inputs are shipped to the device in bf16 (halves HBM+relay traffic; costs
~2e-4 relative error on the final scalar, vs the 2e-2 gate).
"""

import numpy as np
import ml_dtypes

import concourse.bass as bass
import concourse.bacc as bacc
import concourse.tile as tile
from concourse import mybir
from concourse.bass_utils import run_bass_kernel_spmd

AOP = mybir.AluOpType
ACTF = mybir.ActivationFunctionType

# problem constants (hardcoded per harness contract)
FULL_B, CH, H, W = 16, 3, 512, 512
N_CORES = 8
B_LOC = FULL_B // N_CORES  # 2 images per core
C1 = 0.01 ** 2
C2 = 0.03 ** 2
WS = 11
SIGMA = 1.5

# conv chunking: output chunks of 118 rows; input chunks of <=128 rows with 5-halo
CHUNK = 118
N_CH = 5  # ceil(512/118)
# per chunk: (input row start, input rows, output row start, output rows)
CH_IN0 = [0, 113, 231, 349, 467]
CH_INN = [123, 128, 128, 128, 45]
CH_OUT0 = [0, 118, 236, 354, 472]
CH_OUTN = [118, 118, 118, 118, 40]

BF16 = mybir.dt.bfloat16
F32 = mybir.dt.float32

SQRT2 = float(np.sqrt(2.0))


def _gauss():
    """Gaussian taps, ULP-adjusted in bf16 so the bf16 window sums to 1.

    Raw bf16 rounding makes the window gain 0.99919, which biases every
    conv output by -0.08% and the final DSSIM by ~5e-3 relative. Nudging
    taps by +/-1 bf16 ULP (greedy, large taps first) recovers sum == 1
    exactly; measured end-to-end error is ~3.5e-4.
    """
    bf = ml_dtypes.bfloat16
    xs = np.arange(WS) - WS // 2
    g = np.exp(-(xs.astype(np.float64) ** 2) / (2.0 * SIGMA ** 2))
    g = (g / g.sum()).astype(np.float32)
    cand = g.astype(bf)
    for _ in range(4):
        for i in np.argsort(-g):
            base = cand.astype(np.float64).sum() - float(cand[i])
            u = np.array(cand[i], dtype=bf).view(np.uint16)
            opts = [
                np.array(u - 1, dtype=np.uint16).view(bf),
                cand[i],
                np.array(u + 1, dtype=np.uint16).view(bf),
            ]
            errs = [abs(base + float(o) - 1.0) for o in opts]
            cand[i] = opts[int(np.argmin(errs))]
    return cand.astype(np.float32)


def _g2(t, g):
    return g[t + 5] if abs(t) <= 5 else 0.0


def _band_mats():
    """Overlap-save band matrices, used by pass1 (as rhs) and, halved, by
    pass2 (as lhsT).

    mid  [128, 118]: M[j, i] = g(j - i - 5)   (input row = out_row - 5 + j)
    first[123, 118]: M[j, i] = g(j - i)       (rows clipped at image top)
    last [ 45,  40]: M[j, i] = g(j - i - 5)
    """
    g = _gauss()
    mid = np.zeros((128, 118), np.float32)
    for j in range(128):
        for i in range(118):
            mid[j, i] = _g2(j - i - 5, g)
    first = np.zeros((123, 118), np.float32)
    for j in range(123):
        for i in range(118):
            first[j, i] = _g2(j - i, g)
    last = np.zeros((45, 40), np.float32)
    for j in range(45):
        for i in range(40):
            last[j, i] = _g2(j - i - 5, g)
    return first, mid, last


def _act_recip(nc, out, in_):
    """activation(func=Reciprocal) without bass's precision guard."""
    eng = nc.scalar
    return eng.add_instruction(
        mybir.InstActivation(
            name=nc.get_next_instruction_name(),
            func=ACTF.Reciprocal,
            ins=[
                eng.lower_ap(in_),
                mybir.ImmediateValue(dtype=mybir.dt.float32, value=0.0),
                mybir.ImmediateValue(dtype=mybir.dt.float32, value=1.0),
                mybir.ImmediateValue(dtype=mybir.dt.float32, value=0.0),
            ],
            outs=[eng.lower_ap(out)],
        )
    )


def build_bass(n_sets=B_LOC * CH):
    nc = bacc.Bacc("TRN2", target_bir_lowering=False, debug=False)

    x_d = nc.dram_tensor("x", [B_LOC, CH, H, W], BF16, kind="ExternalInput")
    y_d = nc.dram_tensor("y", [B_LOC, CH, H, W], BF16, kind="ExternalInput")
    gf_d = nc.dram_tensor("gf", [123, 118], BF16, kind="ExternalInput")
    gm_d = nc.dram_tensor("gm", [128, 118], BF16, kind="ExternalInput")
    gl_d = nc.dram_tensor("gl", [45, 40], BF16, kind="ExternalInput")
    ghf_d = nc.dram_tensor("ghf", [123, 118], BF16, kind="ExternalInput")
    ghm_d = nc.dram_tensor("ghm", [128, 118], BF16, kind="ExternalInput")
    ghl_d = nc.dram_tensor("ghl", [45, 40], BF16, kind="ExternalInput")
    g4f_d = nc.dram_tensor("g4f", [123, 118], BF16, kind="ExternalInput")
    g4m_d = nc.dram_tensor("g4m", [128, 118], BF16, kind="ExternalInput")
    g4l_d = nc.dram_tensor("g4l", [45, 40], BF16, kind="ExternalInput")
    acc_d = nc.dram_tensor("acc", [128, 1], F32, kind="ExternalOutput")

    with tile.TileContext(nc) as tc:
        with (
            tc.tile_pool(name="consts", bufs=1) as consts,
            tc.tile_pool(name="inp", bufs=4) as inp,
            tc.tile_pool(name="prep", bufs=3) as prep,
            tc.tile_pool(name="t1", bufs=3) as t1p,
            tc.tile_pool(name="mapt", bufs=3) as mapt,
            tc.tile_pool(name="p1", bufs=2, space="PSUM") as p1p,
            tc.tile_pool(name="p2", bufs=2, space="PSUM") as p2p,
        ):
            gf = consts.tile([123, 118], BF16, tag="gf", name="gf")
            nc.sync.dma_start(out=gf, in_=gf_d[:, :])
            gm = consts.tile([128, 118], BF16, tag="gm", name="gm")
            nc.sync.dma_start(out=gm, in_=gm_d[:, :])
            gl = consts.tile([45, 40], BF16, tag="gl", name="gl")
            nc.sync.dma_start(out=gl, in_=gl_d[:, :])
            ghf = consts.tile([123, 118], BF16, tag="ghf", name="ghf")
            nc.sync.dma_start(out=ghf, in_=ghf_d[:, :])
            ghm = consts.tile([128, 118], BF16, tag="ghm", name="ghm")
            nc.sync.dma_start(out=ghm, in_=ghm_d[:, :])
            ghl = consts.tile([45, 40], BF16, tag="ghl", name="ghl")
            nc.sync.dma_start(out=ghl, in_=ghl_d[:, :])
            g4f = consts.tile([123, 118], BF16, tag="g4f", name="g4f")
            nc.sync.dma_start(out=g4f, in_=g4f_d[:, :])
            g4m = consts.tile([128, 118], BF16, tag="g4m", name="g4m")
            nc.sync.dma_start(out=g4m, in_=g4m_d[:, :])
            g4l = consts.tile([45, 40], BF16, tag="g4l", name="g4l")
            nc.sync.dma_start(out=g4l, in_=g4l_d[:, :])

            def gpos(c):
                return (gf, gm, gl)[0 if c == 0 else (2 if c == N_CH - 1 else 1)]

            def ghalf(c):
                return (ghf, ghm, ghl)[0 if c == 0 else (2 if c == N_CH - 1 else 1)]

            def gquad(c):
                return (g4f, g4m, g4l)[0 if c == 0 else (2 if c == N_CH - 1 else 1)]

            acc = consts.tile([128, 1], F32, tag="acc", name="acc")
            nc.vector.memset(acc, 0.0)
            rsums = consts.tile([128, 32], F32, tag="rsums", name="rsums")
            nc.vector.memset(rsums, 0.0)
            iround = 0

            for iset in range(n_sets):
                b, c = divmod(iset, CH)
                # ---- load x, y in 5 overlapped row-chunks: [128, 5*512] bf16
                xo = inp.tile([128, N_CH * W], BF16, tag="xo", name="xo")
                yo = inp.tile([128, N_CH * W], BF16, tag="yo", name="yo")
                # NOTE: halo rows (beyond each chunk's valid row count) are
                # left as garbage on purpose: prep processes them but no
                # consumer (pass1 lhsT, evac, map) ever reads those rows.
                for k in range(N_CH):
                    r0, nr = CH_IN0[k], CH_INN[k]
                    nc.sync.dma_start(
                        out=xo[0:nr, W * k : W * k + W],
                        in_=x_d[b, c, r0 : r0 + nr, :],
                    )
                    # y goes through the Activation HWDGE queue so input
                    # loads spread over twice the physical DMA queues
                    nc.scalar.dma_start(
                        out=yo[0:nr, W * k : W * k + W],
                        in_=y_d[b, c, r0 : r0 + nr, :],
                    )

                # ---- prep: s, d (vector), r = x*y (gpsimd), 2x^2 / 2y^2
                # (scalar Squares, scale sqrt2), q = 2x^2+2y^2 (gpsimd).
                # All depend only on xo/yo. For the first set, run in
                # 512-column chunks gated on the individual chunk DMAs so
                # the pipeline fills while the rest of the load streams in.
                st = prep.tile([128, N_CH * W], BF16, tag="s", name="s")
                dt = prep.tile([128, N_CH * W], BF16, tag="d", name="d")
                rt = prep.tile([128, N_CH * W], BF16, tag="r", name="r")
                qt = prep.tile([128, N_CH * W], BF16, tag="q", name="q")
                x2t = prep.tile([128, N_CH * W], BF16, tag="x2", name="x2")
                y2t = prep.tile([128, N_CH * W], BF16, tag="y2", name="y2")
                for k in range(N_CH):
                    sl = slice(W * k, W * k + W)
                    nc.gpsimd.tensor_mul(rt[:, sl], xo[:, sl], yo[:, sl])
                if iset == 0:
                    for k in range(N_CH):
                        sl = slice(W * k, W * k + W)
                        nc.vector.tensor_add(st[:, sl], xo[:, sl], yo[:, sl])
                        nc.vector.tensor_sub(dt[:, sl], xo[:, sl], yo[:, sl])
                        nc.scalar.activation(
                            out=x2t[:, sl], in_=xo[:, sl],
                            func=ACTF.Square, scale=SQRT2,
                        )
                        nc.scalar.activation(
                            out=y2t[:, sl], in_=yo[:, sl],
                            func=ACTF.Square, scale=SQRT2,
                        )
                else:
                    nc.vector.tensor_add(st, xo, yo)
                    nc.vector.tensor_sub(dt, xo, yo)
                    nc.scalar.activation(
                        out=x2t, in_=xo, func=ACTF.Square, scale=SQRT2
                    )
                    nc.scalar.activation(
                        out=y2t, in_=yo, func=ACTF.Square, scale=SQRT2
                    )
                for k in range(N_CH):
                    sl = slice(W * k, W * k + W)
                    nc.gpsimd.tensor_add(qt[:, sl], x2t[:, sl], y2t[:, sl])

                # ---- per 118-row w-chunk
                for m in range(N_CH):
                    w0, pw = CH_IN0[m], CH_INN[m]
                    kin2, p2 = CH_INN[m], CH_OUTN[m]
                    lgh = ghalf(m)

                    # pass1 half 0: [conv_H(s), conv_H(d)] -> ps1a
                    ps1a = p1p.tile([128, 2, W], F32, tag="p1", name="ps1a")
                    for hi, srcm in ((0, st), (1, dt)):
                        for k in range(N_CH):
                            kin = CH_INN[k]
                            o0, on = CH_OUT0[k], CH_OUTN[k]
                            nc.tensor.matmul(
                                ps1a[0:pw, hi, o0 : o0 + on],
                                lhsT=srcm[0:kin, W * k + w0 : W * k + w0 + pw],
                                rhs=gpos(k)[0:kin, 0:on],
                                start=(k == 0),
                                stop=(k == N_CH - 1),
                            )
                    # evacuate as plain bf16 cast (gpsimd cannot touch PSUM;
                    # alternate scalar/vector so each chunk costs one of each)
                    t1sd = t1p.tile([128, 2, W], BF16, tag="t1sd", name="t1sd")
                    nc.scalar.activation(
                        out=t1sd[0:pw, :, :], in_=ps1a[0:pw, :, :],
                        func=ACTF.Copy,
                    )

                    # pass1 half 1: [conv_H(4xy), conv_H(q)] -> ps1b
                    ps1b = p1p.tile([128, 2, W], F32, tag="p1", name="ps1b")
                    for hi, srcm, bands in ((0, rt, gquad), (1, qt, gpos)):
                        for k in range(N_CH):
                            kin = CH_INN[k]
                            o0, on = CH_OUT0[k], CH_OUTN[k]
                            nc.tensor.matmul(
                                ps1b[0:pw, hi, o0 : o0 + on],
                                lhsT=srcm[0:kin, W * k + w0 : W * k + w0 + pw],
                                rhs=bands(k)[0:kin, 0:on],
                                start=(k == 0),
                                stop=(k == N_CH - 1),
                            )
                    t1pm = t1p.tile([128, 2, W], BF16, tag="t1pm", name="t1pm")
                    if m == 3:
                        nc.vector.tensor_copy(
                            out=t1pm[0:pw, :, :], in_=ps1b[0:pw, :, :]
                        )
                    else:
                        nc.scalar.activation(
                            out=t1pm[0:pw, :, :], in_=ps1b[0:pw, :, :],
                            func=ACTF.Copy,
                        )

                    # pass2: four matmuls (a matmul output must fit one PSUM
                    # bank, 512 f32), all sharing the same half-band lhsT
                    psA = p2p.tile([118, 2, W], F32, tag="psAB", name="psA")
                    for hi in range(2):
                        nc.tensor.matmul(
                            psA[0:p2, hi, :], lhsT=lgh[0:kin2, 0:p2],
                            rhs=t1sd[0:kin2, hi, :], start=True, stop=True,
                        )
                    psB = p2p.tile([118, 2, W], F32, tag="psAB", name="psB")
                    for hi in range(2):
                        nc.tensor.matmul(
                            psB[0:p2, hi, :], lhsT=lgh[0:kin2, 0:p2],
                            rhs=t1pm[0:kin2, hi, :], start=True, stop=True,
                        )

                    # map stage
                    # ab = (psA * sqrt2)^2 = [S^2/2, D^2/2]
                    ab = mapt.tile([118, 2, W], BF16, tag="ab", name="ab")
                    nc.scalar.activation(
                        out=ab[0:p2, :, :], in_=psA[0:p2, :, :],
                        func=ACTF.Square, scale=SQRT2,
                    )
                    # uv = [u + C1, v + C1]: C1 rides on the ab0 term so it
                    # survives both the difference and the sum (STT runs at
                    # DVE 1x mode, but folding C1 here turns numden into a
                    # plain tensor_tensor which gets the 2x packed mode)
                    uv = mapt.tile([118, 2, W], BF16, tag="uv", name="uv")
                    nc.vector.scalar_tensor_tensor(
                        out=uv[0:p2, 0, :], in0=ab[0:p2, 0, :], scalar=C1,
                        in1=ab[0:p2, 1, :], op0=AOP.add, op1=AOP.subtract,
                    )
                    nc.vector.scalar_tensor_tensor(
                        out=uv[0:p2, 1, :], in0=ab[0:p2, 0, :], scalar=C1,
                        in1=ab[0:p2, 1, :], op0=AOP.add, op1=AOP.add,
                    )
                    # nd = (psB + C1 + C2) - uv = [w-u, h-v]  (uv carries +C1)
                    nd = mapt.tile([118, 2, W], BF16, tag="nd", name="nd")
                    nc.vector.scalar_tensor_tensor(
                        out=nd[0:p2, :, :], in0=psB[0:p2, :, :],
                        scalar=C1 + C2,
                        in1=uv[0:p2, :, :], op0=AOP.add, op1=AOP.subtract,
                    )
                    # num/den = uv * nd (C1 already folded into uv)
                    numden = mapt.tile(
                        [118, 2, W], BF16, tag="numden", name="numden"
                    )
                    nc.vector.tensor_mul(
                        numden[0:p2, :, :], uv[0:p2, :, :], nd[0:p2, :, :]
                    )
                    rb = mapt.tile([118, W], BF16, tag="rb", name="rb")
                    _act_recip(nc, rb[0:p2, :], numden[0:p2, 1, :])
                    scr = mapt.tile([118, W], BF16, tag="scr", name="scr")
                    nc.vector.scalar_tensor_tensor(
                        out=scr[0:p2, :], in0=numden[0:p2, 0, :], scalar=1.0,
                        in1=rb[0:p2, :], op0=AOP.mult, op1=AOP.mult,
                        accum_out=rsums[0:p2, iround : iround + 1],
                    )
                    iround += 1

            nc.vector.tensor_reduce(
                out=acc, in_=rsums, op=AOP.add, axis=mybir.AxisListType.X
            )
            nc.scalar.dma_start(out=acc_d[:, :], in_=acc)

    nc.finalize()
    return nc


_NC_CACHE = None


def _get_nc():
    global _NC_CACHE
    if _NC_CACHE is None:
        _NC_CACHE = build_bass()
    return _NC_CACHE


def make_in_maps(x: np.ndarray, y: np.ndarray):
    """Shard full f32 inputs into per-core bf16 input maps."""
    bf = ml_dtypes.bfloat16
    x = np.asarray(x).astype(bf)
    y = np.asarray(y).astype(bf)

    first, mid, last = _band_mats()
    consts = {
        "gf": first.astype(bf),
        "gm": mid.astype(bf),
        "gl": last.astype(bf),
        # exact halves / quadruples (pure exponent shifts in bf16)
        "ghf": (first.astype(bf).astype(np.float32) / 2).astype(bf),
        "ghm": (mid.astype(bf).astype(np.float32) / 2).astype(bf),
        "ghl": (last.astype(bf).astype(np.float32) / 2).astype(bf),
        "g4f": (first.astype(bf).astype(np.float32) * 4).astype(bf),
        "g4m": (mid.astype(bf).astype(np.float32) * 4).astype(bf),
        "g4l": (last.astype(bf).astype(np.float32) * 4).astype(bf),
    }

    in_maps = []
    for core in range(N_CORES):
        b0 = core * B_LOC
        in_maps.append(
            {
                "x": np.ascontiguousarray(x[b0 : b0 + B_LOC]),
                "y": np.ascontiguousarray(y[b0 : b0 + B_LOC]),
                **consts,
            }
        )
    return in_maps


def kernel(x: np.ndarray, y: np.ndarray) -> np.ndarray:
    nc = _get_nc()
    in_maps = make_in_maps(x, y)
    res = run_bass_kernel_spmd(nc, in_maps, core_ids=list(range(N_CORES)))
    total = np.float64(0.0)
    for r in res.results:
        total += np.asarray(r["acc"], dtype=np.float64).sum()
    n_pix = FULL_B * CH * H * W
    return np.float32(1.0 - total / n_pix)


if __name__ == "__main__":
    rng = np.random.default_rng(0)
    x = rng.random((FULL_B, CH, H, W), dtype=np.float32)
    y = rng.random((FULL_B, CH, H, W), dtype=np.float32)
    print("kernel:", kernel(x, y))


# revision 22
# speedup vs baseline: 1.0508x; 1.0508x over previous
"""DSSIM loss kernel for Trainium2, 8 NeuronCores, data-parallel over batch.

Math: for each (b, c) 512x512 image pair (x, y):
  s = x + y, d = x - y                       (prep, bf16)
  S = conv(s), D = conv(d), P = conv(s^2), Q = conv(d^2)  (separable 11-tap)
  u = 2*mu1*mu2       = (S^2 - D^2)/2
  v = mu1^2 + mu2^2   = (S^2 + D^2)/2
  w - u = 2*sigma12 + C2       = (P - Q)/2 + C2 - u
  h - v = sigma1 + sigma2 + C2 = (P + Q)/2 + C2 - v
  ssim = (u + C1)(w - u) / ((v + C1)(h - v));  DSSIM = 1 - mean(ssim)

Structure per (b, c) set:
  prep computes s, d (vector) and s^2, d^2 (scalar Squares).
  pass1 (image chunk as PE-stationary operand) convolves H and transposes;
  both PSUM pairs evacuate as plain bf16 casts (an engine op may read only
  ONE PSUM operand, and GPSIMD cannot touch PSUM at all). The quadratic
  pair then forms the +/- basis with two SBUF ops on gpsimd. pass2 is four
  matmuls (a matmul output must fit one PSUM bank) sharing one lhsT:
    psA = (G/2) . [t1s, t1d]  = [S/2, D/2]
    psB = (G/2) . [e-, e+]    = [(P-Q)/2, (P+Q)/2]
  Map stage (one PSUM operand per op; STT runs at DVE 1x mode, plain
  tensor_tensor bf16 gets the 2x packed mode):
    ab  = Square(psA * sqrt2) = [S^2/2, D^2/2]          (scalar)
    uv  = [(ab0+C1) - ab1, (ab0+C1) + ab1] = [u,v]+C1   (vector STT x2;
          C1 rides on the ab0 term so it survives both sum and difference)
    nd  = (psB + C1 + C2) - uv = [w-u, h-v]             (vector STT)
    numden = uv * nd                                    (vector TT, 2x)
    ssim = numden0 * recip(numden1), accumulated per-partition into rsums.
Host reduces the per-core partition sums.

Inputs are shipped to the device in bf16 (halves HBM+relay traffic; costs
~2e-4 relative error on the final scalar, vs the 2e-2 gate).
"""

import numpy as np
import ml_dtypes

import concourse.bass as bass
import concourse.bacc as bacc
import concourse.tile as tile
from concourse import mybir
from concourse.bass_utils import run_bass_kernel_spmd

AOP = mybir.AluOpType
ACTF = mybir.ActivationFunctionType

# problem constants (hardcoded per harness contract)
FULL_B, CH, H, W = 16, 3, 512, 512
N_CORES = 8
B_LOC = FULL_B // N_CORES  # 2 images per core
C1 = 0.01 ** 2
C2 = 0.03 ** 2
WS = 11
SIGMA = 1.5

# conv chunking: output chunks of 118 rows; input chunks of <=128 rows with 5-halo
CHUNK = 118
N_CH = 5  # ceil(512/118)
# per chunk: (input row start, input rows, output row start, output rows)
CH_IN0 = [0, 113, 231, 349, 467]
CH_INN = [123, 128, 128, 128, 45]
CH_OUT0 = [0, 118, 236, 354, 472]
CH_OUTN = [118, 118, 118, 118, 40]

BF16 = mybir.dt.bfloat16
F32 = mybir.dt.float32

SQRT2 = float(np.sqrt(2.0))


def _gauss():
    """Gaussian taps, ULP-adjusted in bf16 so the bf16 window sums to 1.

    Raw bf16 rounding makes the window gain 0.99919, which biases every
    conv output by -0.08% and the final DSSIM by ~5e-3 relative. Nudging
    taps by +/-1 bf16 ULP (greedy, large taps first) recovers sum == 1
    exactly; measured end-to-end error is ~3.5e-4.
    """
    bf = ml_dtypes.bfloat16
    xs = np.arange(WS) - WS // 2
    g = np.exp(-(xs.astype(np.float64) ** 2) / (2.0 * SIGMA ** 2))
    g = (g / g.sum()).astype(np.float32)
    cand = g.astype(bf)
    for _ in range(4):
        for i in np.argsort(-g):
            base = cand.astype(np.float64).sum() - float(cand[i])
            u = np.array(cand[i], dtype=bf).view(np.uint16)
            opts = [
                np.array(u - 1, dtype=np.uint16).view(bf),
                cand[i],
                np.array(u + 1, dtype=np.uint16).view(bf),
            ]
            errs = [abs(base + float(o) - 1.0) for o in opts]
            cand[i] = opts[int(np.argmin(errs))]
    return cand.astype(np.float32)


def _g2(t, g):
    return g[t + 5] if abs(t) <= 5 else 0.0


def _band_mats():
    """Overlap-save band matrices, used by pass1 (as rhs) and, halved, by
    pass2 (as lhsT).

    mid  [128, 118]: M[j, i] = g(j - i - 5)   (input row = out_row - 5 + j)
    first[123, 118]: M[j, i] = g(j - i)       (rows clipped at image top)
    last [ 45,  40]: M[j, i] = g(j - i - 5)
    """
    g = _gauss()
    mid = np.zeros((128, 118), np.float32)
    for j in range(128):
        for i in range(118):
            mid[j, i] = _g2(j - i - 5, g)
    first = np.zeros((123, 118), np.float32)
    for j in range(123):
        for i in range(118):
            first[j, i] = _g2(j - i, g)
    last = np.zeros((45, 40), np.float32)
    for j in range(45):
        for i in range(40):
            last[j, i] = _g2(j - i - 5, g)
    return first, mid, last


def _act_recip(nc, out, in_):
    """activation(func=Reciprocal) without bass's precision guard."""
    eng = nc.scalar
    return eng.add_instruction(
        mybir.InstActivation(
            name=nc.get_next_instruction_name(),
            func=ACTF.Reciprocal,
            ins=[
                eng.lower_ap(in_),
                mybir.ImmediateValue(dtype=mybir.dt.float32, value=0.0),
                mybir.ImmediateValue(dtype=mybir.dt.float32, value=1.0),
                mybir.ImmediateValue(dtype=mybir.dt.float32, value=0.0),
            ],
            outs=[eng.lower_ap(out)],
        )
    )


def build_bass(n_sets=B_LOC * CH):
    nc = bacc.Bacc("TRN2", target_bir_lowering=False, debug=False)

    x_d = nc.dram_tensor("x", [B_LOC, CH, H, W], BF16, kind="ExternalInput")
    y_d = nc.dram_tensor("y", [B_LOC, CH, H, W], BF16, kind="ExternalInput")
    gf_d = nc.dram_tensor("gf", [123, 118], BF16, kind="ExternalInput")
    gm_d = nc.dram_tensor("gm", [128, 118], BF16, kind="ExternalInput")
    gl_d = nc.dram_tensor("gl", [45, 40], BF16, kind="ExternalInput")
    ghf_d = nc.dram_tensor("ghf", [123, 118], BF16, kind="ExternalInput")
    ghm_d = nc.dram_tensor("ghm", [128, 118], BF16, kind="ExternalInput")
    ghl_d = nc.dram_tensor("ghl", [45, 40], BF16, kind="ExternalInput")
    acc_d = nc.dram_tensor("acc", [128, 1], F32, kind="ExternalOutput")

    with tile.TileContext(nc) as tc:
        with (
            tc.tile_pool(name="consts", bufs=1) as consts,
            tc.tile_pool(name="inp", bufs=4) as inp,
            tc.tile_pool(name="prep", bufs=3) as prep,
            tc.tile_pool(name="t1", bufs=3) as t1p,
            tc.tile_pool(name="mapt", bufs=3) as mapt,
            tc.tile_pool(name="p1", bufs=2, space="PSUM") as p1p,
            tc.tile_pool(name="p2", bufs=2, space="PSUM") as p2p,
        ):
            gf = consts.tile([123, 118], BF16, tag="gf", name="gf")
            nc.sync.dma_start(out=gf, in_=gf_d[:, :])
            gm = consts.tile([128, 118], BF16, tag="gm", name="gm")
            nc.sync.dma_start(out=gm, in_=gm_d[:, :])
            gl = consts.tile([45, 40], BF16, tag="gl", name="gl")
            nc.sync.dma_start(out=gl, in_=gl_d[:, :])
            ghf = consts.tile([123, 118], BF16, tag="ghf", name="ghf")
            nc.sync.dma_start(out=ghf, in_=ghf_d[:, :])
            ghm = consts.tile([128, 118], BF16, tag="ghm", name="ghm")
            nc.sync.dma_start(out=ghm, in_=ghm_d[:, :])
            ghl = consts.tile([45, 40], BF16, tag="ghl", name="ghl")
            nc.sync.dma_start(out=ghl, in_=ghl_d[:, :])

            def gpos(c):
                return (gf, gm, gl)[0 if c == 0 else (2 if c == N_CH - 1 else 1)]

            def ghalf(c):
                return (ghf, ghm, ghl)[0 if c == 0 else (2 if c == N_CH - 1 else 1)]

            acc = consts.tile([128, 1], F32, tag="acc", name="acc")
            nc.vector.memset(acc, 0.0)
            rsums = consts.tile([128, 32], F32, tag="rsums", name="rsums")
            nc.vector.memset(rsums, 0.0)
            iround = 0

            for iset in range(n_sets):
                b, c = divmod(iset, CH)
                # ---- load x, y in 5 overlapped row-chunks: [128, 5*512] bf16
                # NOTE: halo rows (beyond each chunk's valid row count) stay
                # garbage on purpose: prep processes them but no consumer
                # (pass1 lhsT, evac, map) ever reads those rows.
                xo = inp.tile([128, N_CH * W], BF16, tag="xo", name="xo")
                yo = inp.tile([128, N_CH * W], BF16, tag="yo", name="yo")
                for k in range(N_CH):
                    r0, nr = CH_IN0[k], CH_INN[k]
                    nc.sync.dma_start(
                        out=xo[0:nr, W * k : W * k + W],
                        in_=x_d[b, c, r0 : r0 + nr, :],
                    )
                    # y goes through the Activation HWDGE queue so input
                    # loads spread over twice the physical DMA queues
                    nc.scalar.dma_start(
                        out=yo[0:nr, W * k : W * k + W],
                        in_=y_d[b, c, r0 : r0 + nr, :],
                    )

                # ---- prep: s, d on vector; s2, d2 as scalar Squares.
                # First set runs in 512-column chunks gated on individual
                # chunk DMAs so the pipeline fills while the load streams.
                st = prep.tile([128, N_CH * W], BF16, tag="s", name="s")
                dt = prep.tile([128, N_CH * W], BF16, tag="d", name="d")
                s2t = prep.tile([128, N_CH * W], BF16, tag="s2", name="s2")
                d2t = prep.tile([128, N_CH * W], BF16, tag="d2", name="d2")
                if iset == 0:
                    for k in range(N_CH):
                        sl = slice(W * k, W * k + W)
                        nc.vector.tensor_add(st[:, sl], xo[:, sl], yo[:, sl])
                        nc.vector.tensor_sub(dt[:, sl], xo[:, sl], yo[:, sl])
                        nc.scalar.activation(
                            out=s2t[:, sl], in_=st[:, sl], func=ACTF.Square
                        )
                        nc.scalar.activation(
                            out=d2t[:, sl], in_=dt[:, sl], func=ACTF.Square
                        )
                else:
                    nc.vector.tensor_add(st, xo, yo)
                    nc.vector.tensor_sub(dt, xo, yo)
                    nc.scalar.activation(out=s2t, in_=st, func=ACTF.Square)
                    nc.scalar.activation(out=d2t, in_=dt, func=ACTF.Square)

                # ---- per 118-row w-chunk
                for m in range(N_CH):
                    w0, pw = CH_IN0[m], CH_INN[m]
                    kin2, p2 = CH_INN[m], CH_OUTN[m]
                    lgh = ghalf(m)

                    # pass1 half 0: [conv_H(s), conv_H(d)] -> ps1a
                    ps1a = p1p.tile([128, 2, W], F32, tag="p1", name="ps1a")
                    for hi, srcm in ((0, st), (1, dt)):
                        for k in range(N_CH):
                            kin = CH_INN[k]
                            o0, on = CH_OUT0[k], CH_OUTN[k]
                            nc.tensor.matmul(
                                ps1a[0:pw, hi, o0 : o0 + on],
                                lhsT=srcm[0:kin, W * k + w0 : W * k + w0 + pw],
                                rhs=gpos(k)[0:kin, 0:on],
                                start=(k == 0),
                                stop=(k == N_CH - 1),
                            )
                    t1sd = t1p.tile([128, 2, W], BF16, tag="t1sd", name="t1sd")
                    nc.scalar.activation(
                        out=t1sd[0:pw, :, :], in_=ps1a[0:pw, :, :],
                        func=ACTF.Copy,
                    )

                    # pass1 half 1: [conv_H(s2), conv_H(d2)] -> ps1b
                    ps1b = p1p.tile([128, 2, W], F32, tag="p1", name="ps1b")
                    for hi, srcm in ((0, s2t), (1, d2t)):
                        for k in range(N_CH):
                            kin = CH_INN[k]
                            o0, on = CH_OUT0[k], CH_OUTN[k]
                            nc.tensor.matmul(
                                ps1b[0:pw, hi, o0 : o0 + on],
                                lhsT=srcm[0:kin, W * k + w0 : W * k + w0 + pw],
                                rhs=gpos(k)[0:kin, 0:on],
                                start=(k == 0),
                                stop=(k == N_CH - 1),
                            )
                    # evacuate plainly, then form the +/- basis in SBUF
                    pq = t1p.tile([128, 2, W], BF16, tag="pq", name="pq")
                    if m in (1, 4):
                        nc.scalar.activation(
                            out=pq[0:pw, :, :], in_=ps1b[0:pw, :, :],
                            func=ACTF.Copy,
                        )
                    else:
                        nc.vector.tensor_copy(
                            out=pq[0:pw, :, :], in_=ps1b[0:pw, :, :]
                        )
                    t1pm = t1p.tile([128, 2, W], BF16, tag="t1pm", name="t1pm")
                    nc.gpsimd.tensor_sub(
                        t1pm[0:pw, 0, :], pq[0:pw, 0, :], pq[0:pw, 1, :]
                    )
                    nc.gpsimd.tensor_add(
                        t1pm[0:pw, 1, :], pq[0:pw, 0, :], pq[0:pw, 1, :]
                    )

                    # pass2: four matmuls (a matmul output must fit one PSUM
                    # bank, 512 f32), all sharing the same half-band lhsT
                    psA = p2p.tile([118, 2, W], F32, tag="psAB", name="psA")
                    for hi in range(2):
                        nc.tensor.matmul(
                            psA[0:p2, hi, :], lhsT=lgh[0:kin2, 0:p2],
                            rhs=t1sd[0:kin2, hi, :], start=True, stop=True,
                        )
                    psB = p2p.tile([118, 2, W], F32, tag="psAB", name="psB")
                    for hi in range(2):
                        nc.tensor.matmul(
                            psB[0:p2, hi, :], lhsT=lgh[0:kin2, 0:p2],
                            rhs=t1pm[0:kin2, hi, :], start=True, stop=True,
                        )

                    # map stage
                    # ab = (psA * sqrt2)^2 = [S^2/2, D^2/2]
                    ab = mapt.tile([118, 2, W], BF16, tag="ab", name="ab")
                    nc.scalar.activation(
                        out=ab[0:p2, :, :], in_=psA[0:p2, :, :],
                        func=ACTF.Square, scale=SQRT2,
                    )
                    # uv = [u + C1, v + C1]: C1 rides on the ab0 term so it
                    # survives both the difference and the sum
                    uv = mapt.tile([118, 2, W], BF16, tag="uv", name="uv")
                    nc.vector.scalar_tensor_tensor(
                        out=uv[0:p2, 0, :], in0=ab[0:p2, 0, :], scalar=C1,
                        in1=ab[0:p2, 1, :], op0=AOP.add, op1=AOP.subtract,
                    )
                    nc.vector.scalar_tensor_tensor(
                        out=uv[0:p2, 1, :], in0=ab[0:p2, 0, :], scalar=C1,
                        in1=ab[0:p2, 1, :], op0=AOP.add, op1=AOP.add,
                    )
                    # nd = (psB + C1 + C2) - uv = [w-u, h-v]  (uv carries +C1)
                    nd = mapt.tile([118, 2, W], BF16, tag="nd", name="nd")
                    nc.vector.scalar_tensor_tensor(
                        out=nd[0:p2, :, :], in0=psB[0:p2, :, :],
                        scalar=C1 + C2,
                        in1=uv[0:p2, :, :], op0=AOP.add, op1=AOP.subtract,
                    )
                    # num/den = uv * nd (C1 already folded into uv)
                    numden = mapt.tile(
                        [118, 2, W], BF16, tag="numden", name="numden"
                    )
                    nc.vector.tensor_mul(
                        numden[0:p2, :, :], uv[0:p2, :, :], nd[0:p2, :, :]
                    )
                    rb = mapt.tile([118, W], BF16, tag="rb", name="rb")
                    _act_recip(nc, rb[0:p2, :], numden[0:p2, 1, :])
                    scr = mapt.tile([118, W], BF16, tag="scr", name="scr")
                    nc.vector.scalar_tensor_tensor(
                        out=scr[0:p2, :], in0=numden[0:p2, 0, :], scalar=1.0,
                        in1=rb[0:p2, :], op0=AOP.mult, op1=AOP.mult,
                        accum_out=rsums[0:p2, iround : iround + 1],
                    )
                    iround += 1

            nc.vector.tensor_reduce(
                out=acc, in_=rsums, op=AOP.add, axis=mybir.AxisListType.X
            )
            nc.scalar.dma_start(out=acc_d[:, :], in_=acc)

    nc.finalize()
    return nc


_NC_CACHE = None


def _get_nc():
    global _NC_CACHE
    if _NC_CACHE is None:
        _NC_CACHE = build_bass()
    return _NC_CACHE


def make_in_maps(x: np.ndarray, y: np.ndarray):
    """Shard full f32 inputs into per-core bf16 input maps."""
    bf = ml_dtypes.bfloat16
    x = np.asarray(x).astype(bf)
    y = np.asarray(y).astype(bf)

    first, mid, last = _band_mats()
    consts = {
        "gf": first.astype(bf),
        "gm": mid.astype(bf),
        "gl": last.astype(bf),
        # exact halves (pure exponent shift in bf16)
        "ghf": (first.astype(bf).astype(np.float32) / 2).astype(bf),
        "ghm": (mid.astype(bf).astype(np.float32) / 2).astype(bf),
        "ghl": (last.astype(bf).astype(np.float32) / 2).astype(bf),
    }

    in_maps = []
    for core in range(N_CORES):
        b0 = core * B_LOC
        in_maps.append(
            {
                "x": np.ascontiguousarray(x[b0 : b0 + B_LOC]),
                "y": np.ascontiguousarray(y[b0 : b0 + B_LOC]),
                **consts,
            }
        )
    return in_maps


def kernel(x: np.ndarray, y: np.ndarray) -> np.ndarray:
    nc = _get_nc()
    in_maps = make_in_maps(x, y)
    res = run_bass_kernel_spmd(nc, in_maps, core_ids=list(range(N_CORES)))
    total = np.float64(0.0)
    for r in res.results:
        total += np.asarray(r["acc"], dtype=np.float64).sum()
    n_pix = FULL_B * CH * H * W
    return np.float32(1.0 - total / n_pix)


if __name__ == "__main__":
    rng = np.random.default_rng(0)
    x = rng.random((FULL_B, CH, H, W), dtype=np.float32)
    y = rng.random((FULL_B, CH, H, W), dtype=np.float32)
    print("kernel:", kernel(x, y))


# revision 24
# speedup vs baseline: 1.0929x; 1.0401x over previous
"""DSSIM loss kernel for Trainium2, 8 NeuronCores, data-parallel over batch.

Math: for each (b, c) 512x512 image pair (x, y):
  s = x + y, d = x - y                       (prep, bf16)
  S = conv(s), D = conv(d), P = conv(s^2), Q = conv(d^2)  (separable 11-tap)
  u = 2*mu1*mu2       = (S^2 - D^2)/2
  v = mu1^2 + mu2^2   = (S^2 + D^2)/2
  w - u = 2*sigma12 + C2       = (P - Q)/2 + C2 - u
  h - v = sigma1 + sigma2 + C2 = (P + Q)/2 + C2 - v
  ssim = (u + C1)(w - u) / ((v + C1)(h - v));  DSSIM = 1 - mean(ssim)

Structure per (b, c) set:
  prep computes s, d (vector) and s^2, d^2 (scalar Squares).
  pass1 (image chunk as PE-stationary operand) convolves H and transposes;
  both PSUM pairs evacuate as plain bf16 casts (an engine op may read only
  ONE PSUM operand, and GPSIMD cannot touch PSUM at all). The quadratic
  pair then forms the +/- basis with two SBUF ops on gpsimd. pass2 is four
  matmuls (a matmul output must fit one PSUM bank) sharing one lhsT:
    psA = (G/2) . [t1s, t1d]  = [S/2, D/2]
    psB = (G/2) . [e-, e+]    = [(P-Q)/2, (P+Q)/2]
  Map stage (one PSUM operand per op; STT runs at DVE 1x mode, plain
  tensor_tensor bf16 gets the 2x packed mode):
    ab  = Square(psA * sqrt2) = [S^2/2, D^2/2]          (scalar)
    uv  = [(ab0+C1) - ab1, (ab0+C1) + ab1] = [u,v]+C1   (vector STT x2;
          C1 rides on the ab0 term so it survives both sum and difference)
    nd  = (psB + C1 + C2) - uv = [w-u, h-v]             (vector STT)
    numden = uv * nd                                    (vector TT, 2x)
    ssim = numden0 * recip(numden1), accumulated per-partition into rsums.
Host reduces the per-core partition sums.

Inputs are shipped to the device in bf16 (halves HBM+relay traffic; costs
~2e-4 relative error on the final scalar, vs the 2e-2 gate).
"""

import numpy as np
import ml_dtypes

import concourse.bass as bass
import concourse.bacc as bacc
import concourse.tile as tile
from concourse import mybir
from concourse.bass_utils import run_bass_kernel_spmd

AOP = mybir.AluOpType
ACTF = mybir.ActivationFunctionType

# problem constants (hardcoded per harness contract)
FULL_B, CH, H, W = 16, 3, 512, 512
N_CORES = 8
B_LOC = FULL_B // N_CORES  # 2 images per core
C1 = 0.01 ** 2
C2 = 0.03 ** 2
WS = 11
SIGMA = 1.5

# conv chunking: output chunks of 118 rows; input chunks of <=128 rows with 5-halo
CHUNK = 118
N_CH = 5  # ceil(512/118)
# per chunk: (input row start, input rows, output row start, output rows)
CH_IN0 = [0, 113, 231, 349, 467]
CH_INN = [123, 128, 128, 128, 45]
CH_OUT0 = [0, 118, 236, 354, 472]
CH_OUTN = [118, 118, 118, 118, 40]

BF16 = mybir.dt.bfloat16
F32 = mybir.dt.float32

SQRT2 = float(np.sqrt(2.0))


def _gauss():
    """Gaussian taps, ULP-adjusted in bf16 so the bf16 window sums to 1.

    Raw bf16 rounding makes the window gain 0.99919, which biases every
    conv output by -0.08% and the final DSSIM by ~5e-3 relative. Nudging
    taps by +/-1 bf16 ULP (greedy, large taps first) recovers sum == 1
    exactly; measured end-to-end error is ~3.5e-4.
    """
    bf = ml_dtypes.bfloat16
    xs = np.arange(WS) - WS // 2
    g = np.exp(-(xs.astype(np.float64) ** 2) / (2.0 * SIGMA ** 2))
    g = (g / g.sum()).astype(np.float32)
    cand = g.astype(bf)
    for _ in range(4):
        for i in np.argsort(-g):
            base = cand.astype(np.float64).sum() - float(cand[i])
            u = np.array(cand[i], dtype=bf).view(np.uint16)
            opts = [
                np.array(u - 1, dtype=np.uint16).view(bf),
                cand[i],
                np.array(u + 1, dtype=np.uint16).view(bf),
            ]
            errs = [abs(base + float(o) - 1.0) for o in opts]
            cand[i] = opts[int(np.argmin(errs))]
    return cand.astype(np.float32)


def _g2(t, g):
    return g[t + 5] if abs(t) <= 5 else 0.0


def _band_mats():
    """Overlap-save band matrices, used by pass1 (as rhs) and, halved, by
    pass2 (as lhsT).

    mid  [128, 118]: M[j, i] = g(j - i - 5)   (input row = out_row - 5 + j)
    first[123, 118]: M[j, i] = g(j - i)       (rows clipped at image top)
    last [ 45,  40]: M[j, i] = g(j - i - 5)
    """
    g = _gauss()
    mid = np.zeros((128, 118), np.float32)
    for j in range(128):
        for i in range(118):
            mid[j, i] = _g2(j - i - 5, g)
    first = np.zeros((123, 118), np.float32)
    for j in range(123):
        for i in range(118):
            first[j, i] = _g2(j - i, g)
    last = np.zeros((45, 40), np.float32)
    for j in range(45):
        for i in range(40):
            last[j, i] = _g2(j - i - 5, g)
    return first, mid, last


def _act_recip(nc, out, in_):
    """activation(func=Reciprocal) without bass's precision guard."""
    eng = nc.scalar
    return eng.add_instruction(
        mybir.InstActivation(
            name=nc.get_next_instruction_name(),
            func=ACTF.Reciprocal,
            ins=[
                eng.lower_ap(in_),
                mybir.ImmediateValue(dtype=mybir.dt.float32, value=0.0),
                mybir.ImmediateValue(dtype=mybir.dt.float32, value=1.0),
                mybir.ImmediateValue(dtype=mybir.dt.float32, value=0.0),
            ],
            outs=[eng.lower_ap(out)],
        )
    )


def build_bass(n_sets=B_LOC * CH):
    nc = bacc.Bacc("TRN2", target_bir_lowering=False, debug=False)

    x_d = nc.dram_tensor("x", [B_LOC, CH, H, W], BF16, kind="ExternalInput")
    y_d = nc.dram_tensor("y", [B_LOC, CH, H, W], BF16, kind="ExternalInput")
    gf_d = nc.dram_tensor("gf", [123, 118], BF16, kind="ExternalInput")
    gm_d = nc.dram_tensor("gm", [128, 118], BF16, kind="ExternalInput")
    gl_d = nc.dram_tensor("gl", [45, 40], BF16, kind="ExternalInput")
    ghf_d = nc.dram_tensor("ghf", [123, 118], BF16, kind="ExternalInput")
    ghm_d = nc.dram_tensor("ghm", [128, 118], BF16, kind="ExternalInput")
    ghl_d = nc.dram_tensor("ghl", [45, 40], BF16, kind="ExternalInput")
    acc_d = nc.dram_tensor("acc", [128, 1], F32, kind="ExternalOutput")

    with tile.TileContext(nc) as tc:
        with (
            tc.tile_pool(name="consts", bufs=1) as consts,
            tc.tile_pool(name="inp", bufs=4) as inp,
            tc.tile_pool(name="prep", bufs=3) as prep,
            tc.tile_pool(name="t1", bufs=3) as t1p,
            tc.tile_pool(name="mapt", bufs=3) as mapt,
            tc.tile_pool(name="p1", bufs=2, space="PSUM") as p1p,
            tc.tile_pool(name="p2", bufs=2, space="PSUM") as p2p,
        ):
            gf = consts.tile([123, 118], BF16, tag="gf", name="gf")
            nc.sync.dma_start(out=gf, in_=gf_d[:, :])
            gm = consts.tile([128, 118], BF16, tag="gm", name="gm")
            nc.sync.dma_start(out=gm, in_=gm_d[:, :])
            gl = consts.tile([45, 40], BF16, tag="gl", name="gl")
            nc.sync.dma_start(out=gl, in_=gl_d[:, :])
            ghf = consts.tile([123, 118], BF16, tag="ghf", name="ghf")
            nc.sync.dma_start(out=ghf, in_=ghf_d[:, :])
            ghm = consts.tile([128, 118], BF16, tag="ghm", name="ghm")
            nc.sync.dma_start(out=ghm, in_=ghm_d[:, :])
            ghl = consts.tile([45, 40], BF16, tag="ghl", name="ghl")
            nc.sync.dma_start(out=ghl, in_=ghl_d[:, :])

            def gpos(c):
                return (gf, gm, gl)[0 if c == 0 else (2 if c == N_CH - 1 else 1)]

            def ghalf(c):
                return (ghf, ghm, ghl)[0 if c == 0 else (2 if c == N_CH - 1 else 1)]

            acc = consts.tile([128, 1], F32, tag="acc", name="acc")
            nc.vector.memset(acc, 0.0)
            rsums = consts.tile([128, 32], F32, tag="rsums", name="rsums")
            nc.vector.memset(rsums, 0.0)
            iround = 0

            for iset in range(n_sets):
                b, c = divmod(iset, CH)
                # ---- load x, y in 5 overlapped row-chunks: [128, 5*512] bf16
                # NOTE: halo rows (beyond each chunk's valid row count) stay
                # garbage on purpose: prep processes them but no consumer
                # (pass1 lhsT, evac, map) ever reads those rows.
                xo = inp.tile([128, N_CH * W], BF16, tag="xo", name="xo")
                yo = inp.tile([128, N_CH * W], BF16, tag="yo", name="yo")
                for k in range(N_CH):
                    r0, nr = CH_IN0[k], CH_INN[k]
                    # split each chunk load into two row-halves: one
                    # dma_start's descriptors serialize on a single queue
                    # (~80ns/row), so halving doubles queue parallelism.
                    # y goes through the Activation HWDGE queue family so
                    # loads spread over twice the physical DMA queues.
                    h1 = nr // 2
                    for a, bnd in ((0, h1), (h1, nr)):
                        nc.sync.dma_start(
                            out=xo[a:bnd, W * k : W * k + W],
                            in_=x_d[b, c, r0 + a : r0 + bnd, :],
                        )
                        nc.scalar.dma_start(
                            out=yo[a:bnd, W * k : W * k + W],
                            in_=y_d[b, c, r0 + a : r0 + bnd, :],
                        )

                # ---- prep: s, d on vector; s2, d2 as scalar Squares.
                # First set runs in 512-column chunks gated on individual
                # chunk DMAs so the pipeline fills while the load streams.
                st = prep.tile([128, N_CH * W], BF16, tag="s", name="s")
                dt = prep.tile([128, N_CH * W], BF16, tag="d", name="d")
                s2t = prep.tile([128, N_CH * W], BF16, tag="s2", name="s2")
                d2t = prep.tile([128, N_CH * W], BF16, tag="d2", name="d2")
                if iset == 0:
                    for k in range(N_CH):
                        sl = slice(W * k, W * k + W)
                        nc.vector.tensor_add(st[:, sl], xo[:, sl], yo[:, sl])
                        nc.vector.tensor_sub(dt[:, sl], xo[:, sl], yo[:, sl])
                        nc.scalar.activation(
                            out=s2t[:, sl], in_=st[:, sl], func=ACTF.Square
                        )
                        nc.scalar.activation(
                            out=d2t[:, sl], in_=dt[:, sl], func=ACTF.Square
                        )
                else:
                    nc.vector.tensor_add(st, xo, yo)
                    nc.vector.tensor_sub(dt, xo, yo)
                    nc.scalar.activation(out=s2t, in_=st, func=ACTF.Square)
                    nc.scalar.activation(out=d2t, in_=dt, func=ACTF.Square)

                # ---- per 118-row w-chunk
                for m in range(N_CH):
                    w0, pw = CH_IN0[m], CH_INN[m]
                    kin2, p2 = CH_INN[m], CH_OUTN[m]
                    lgh = ghalf(m)

                    # pass1 half 0: [conv_H(s), conv_H(d)] -> ps1a
                    ps1a = p1p.tile([128, 2, W], F32, tag="p1", name="ps1a")
                    for hi, srcm in ((0, st), (1, dt)):
                        for k in range(N_CH):
                            kin = CH_INN[k]
                            o0, on = CH_OUT0[k], CH_OUTN[k]
                            nc.tensor.matmul(
                                ps1a[0:pw, hi, o0 : o0 + on],
                                lhsT=srcm[0:kin, W * k + w0 : W * k + w0 + pw],
                                rhs=gpos(k)[0:kin, 0:on],
                                start=(k == 0),
                                stop=(k == N_CH - 1),
                            )
                    t1sd = t1p.tile([128, 2, W], BF16, tag="t1sd", name="t1sd")
                    nc.scalar.activation(
                        out=t1sd[0:pw, :, :], in_=ps1a[0:pw, :, :],
                        func=ACTF.Copy,
                    )

                    # pass1 half 1: [conv_H(s2), conv_H(d2)] -> ps1b
                    ps1b = p1p.tile([128, 2, W], F32, tag="p1", name="ps1b")
                    for hi, srcm in ((0, s2t), (1, d2t)):
                        for k in range(N_CH):
                            kin = CH_INN[k]
                            o0, on = CH_OUT0[k], CH_OUTN[k]
                            nc.tensor.matmul(
                                ps1b[0:pw, hi, o0 : o0 + on],
                                lhsT=srcm[0:kin, W * k + w0 : W * k + w0 + pw],
                                rhs=gpos(k)[0:kin, 0:on],
                                start=(k == 0),
                                stop=(k == N_CH - 1),
                            )
                    # evacuate plainly, then form the +/- basis in SBUF
                    pq = t1p.tile([128, 2, W], BF16, tag="pq", name="pq")
                    nc.scalar.activation(
                        out=pq[0:pw, :, :], in_=ps1b[0:pw, :, :],
                        func=ACTF.Copy,
                    )
                    t1pm = t1p.tile([128, 2, W], BF16, tag="t1pm", name="t1pm")
                    nc.gpsimd.tensor_sub(
                        t1pm[0:pw, 0, :], pq[0:pw, 0, :], pq[0:pw, 1, :]
                    )
                    nc.gpsimd.tensor_add(
                        t1pm[0:pw, 1, :], pq[0:pw, 0, :], pq[0:pw, 1, :]
                    )

                    # pass2: four matmuls (a matmul output must fit one PSUM
                    # bank, 512 f32), all sharing the same half-band lhsT
                    psA = p2p.tile([118, 2, W], F32, tag="psAB", name="psA")
                    for hi in range(2):
                        nc.tensor.matmul(
                            psA[0:p2, hi, :], lhsT=lgh[0:kin2, 0:p2],
                            rhs=t1sd[0:kin2, hi, :], start=True, stop=True,
                        )
                    psB = p2p.tile([118, 2, W], F32, tag="psAB", name="psB")
                    for hi in range(2):
                        nc.tensor.matmul(
                            psB[0:p2, hi, :], lhsT=lgh[0:kin2, 0:p2],
                            rhs=t1pm[0:kin2, hi, :], start=True, stop=True,
                        )

                    # map stage
                    # ab = (psA * sqrt2)^2 = [S^2/2, D^2/2]
                    ab = mapt.tile([118, 2, W], BF16, tag="ab", name="ab")
                    nc.scalar.activation(
                        out=ab[0:p2, :, :], in_=psA[0:p2, :, :],
                        func=ACTF.Square, scale=SQRT2,
                    )
                    # uv = [u + C1, v + C1]: C1 rides on the ab0 term so it
                    # survives both the difference and the sum
                    uv = mapt.tile([118, 2, W], BF16, tag="uv", name="uv")
                    nc.vector.scalar_tensor_tensor(
                        out=uv[0:p2, 0, :], in0=ab[0:p2, 0, :], scalar=C1,
                        in1=ab[0:p2, 1, :], op0=AOP.add, op1=AOP.subtract,
                    )
                    nc.vector.scalar_tensor_tensor(
                        out=uv[0:p2, 1, :], in0=ab[0:p2, 0, :], scalar=C1,
                        in1=ab[0:p2, 1, :], op0=AOP.add, op1=AOP.add,
                    )
                    # nd = (psB + C1 + C2) - uv = [w-u, h-v]  (uv carries +C1)
                    nd = mapt.tile([118, 2, W], BF16, tag="nd", name="nd")
                    nc.vector.scalar_tensor_tensor(
                        out=nd[0:p2, :, :], in0=psB[0:p2, :, :],
                        scalar=C1 + C2,
                        in1=uv[0:p2, :, :], op0=AOP.add, op1=AOP.subtract,
                    )
                    # num/den = uv * nd (C1 already folded into uv)
                    numden = mapt.tile(
                        [118, 2, W], BF16, tag="numden", name="numden"
                    )
                    nc.vector.tensor_mul(
                        numden[0:p2, :, :], uv[0:p2, :, :], nd[0:p2, :, :]
                    )
                    rb = mapt.tile([118, W], BF16, tag="rb", name="rb")
                    _act_recip(nc, rb[0:p2, :], numden[0:p2, 1, :])
                    scr = mapt.tile([118, W], BF16, tag="scr", name="scr")
                    nc.vector.scalar_tensor_tensor(
                        out=scr[0:p2, :], in0=numden[0:p2, 0, :], scalar=1.0,
                        in1=rb[0:p2, :], op0=AOP.mult, op1=AOP.mult,
                        accum_out=rsums[0:p2, iround : iround + 1],
                    )
                    iround += 1

            nc.vector.tensor_reduce(
                out=acc, in_=rsums, op=AOP.add, axis=mybir.AxisListType.X
            )
            nc.scalar.dma_start(out=acc_d[:, :], in_=acc)

    nc.finalize()
    return nc


_NC_CACHE = None


def _get_nc():
    global _NC_CACHE
    if _NC_CACHE is None:
        _NC_CACHE = build_bass()
    return _NC_CACHE


def make_in_maps(x: np.ndarray, y: np.ndarray):
    """Shard full f32 inputs into per-core bf16 input maps."""
    bf = ml_dtypes.bfloat16
    x = np.asarray(x).astype(bf)
    y = np.asarray(y).astype(bf)

    first, mid, last = _band_mats()
    consts = {
        "gf": first.astype(bf),
        "gm": mid.astype(bf),
        "gl": last.astype(bf),
        # exact halves (pure exponent shift in bf16)
        "ghf": (first.astype(bf).astype(np.float32) / 2).astype(bf),
        "ghm": (mid.astype(bf).astype(np.float32) / 2).astype(bf),
        "ghl": (last.astype(bf).astype(np.float32) / 2).astype(bf),
    }

    in_maps = []
    for core in range(N_CORES):
        b0 = core * B_LOC
        in_maps.append(
            {
                "x": np.ascontiguousarray(x[b0 : b0 + B_LOC]),
                "y": np.ascontiguousarray(y[b0 : b0 + B_LOC]),
                **consts,
            }
        )
    return in_maps


def kernel(x: np.ndarray, y: np.ndarray) -> np.ndarray:
    nc = _get_nc()
    in_maps = make_in_maps(x, y)
    res = run_bass_kernel_spmd(nc, in_maps, core_ids=list(range(N_CORES)))
    total = np.float64(0.0)
    for r in res.results:
        total += np.asarray(r["acc"], dtype=np.float64).sum()
    n_pix = FULL_B * CH * H * W
    return np.float32(1.0 - total / n_pix)


if __name__ == "__main__":
    rng = np.random.default_rng(0)
    x = rng.random((FULL_B, CH, H, W), dtype=np.float32)
    y = rng.random((FULL_B, CH, H, W), dtype=np.float32)
    print("kernel:", kernel(x, y))


# revision 25
# speedup vs baseline: 1.2422x; 1.1366x over previous
"""DSSIM loss kernel for Trainium2, 8 NeuronCores, data-parallel over batch.

Math: for each (b, c) 512x512 image pair (x, y):
  s = x + y, d = x - y                       (prep, bf16)
  S = conv(s), D = conv(d), P = conv(s^2), Q = conv(d^2)  (separable 11-tap)
  u = 2*mu1*mu2       = (S^2 - D^2)/2
  v = mu1^2 + mu2^2   = (S^2 + D^2)/2
  w - u = 2*sigma12 + C2       = (P - Q)/2 + C2 - u
  h - v = sigma1 + sigma2 + C2 = (P + Q)/2 + C2 - v
  ssim = (u + C1)(w - u) / ((v + C1)(h - v));  DSSIM = 1 - mean(ssim)

Structure per (b, c) set:
  prep computes s, d (vector) and s^2, d^2 (scalar Squares).
  pass1 (image chunk as PE-stationary operand) convolves H and transposes;
  both PSUM pairs evacuate as plain bf16 casts (an engine op may read only
  ONE PSUM operand, and GPSIMD cannot touch PSUM at all). The quadratic
  pair then forms the +/- basis with two SBUF ops on gpsimd. pass2 is four
  matmuls (a matmul output must fit one PSUM bank) sharing one lhsT:
    psA = (G/2) . [t1s, t1d]  = [S/2, D/2]
    psB = (G/2) . [e-, e+]    = [(P-Q)/2, (P+Q)/2]
  Map stage (one PSUM operand per op; STT runs at DVE 1x mode, plain
  tensor_tensor bf16 gets the 2x packed mode):
    ab  = Square(psA * sqrt2) = [S^2/2, D^2/2]          (scalar)
    uv  = [(ab0+C1) - ab1, (ab0+C1) + ab1] = [u,v]+C1   (vector STT x2;
          C1 rides on the ab0 term so it survives both sum and difference)
    nd  = (psB + C1 + C2) - uv = [w-u, h-v]             (vector STT)
    numden = uv * nd                                    (vector TT, 2x)
    ssim = numden0 * recip(numden1), accumulated per-partition into rsums.
Host reduces the per-core partition sums.

Inputs are shipped to the device in bf16 (halves HBM+relay traffic; costs
~2e-4 relative error on the final scalar, vs the 2e-2 gate).
"""

import numpy as np
import ml_dtypes

import concourse.bass as bass
import concourse.bacc as bacc
import concourse.tile as tile
from concourse import mybir
from concourse.bass_utils import run_bass_kernel_spmd

AOP = mybir.AluOpType
ACTF = mybir.ActivationFunctionType

# problem constants (hardcoded per harness contract)
FULL_B, CH, H, W = 16, 3, 512, 512
N_CORES = 8
B_LOC = FULL_B // N_CORES  # 2 images per core
C1 = 0.01 ** 2
C2 = 0.03 ** 2
WS = 11
SIGMA = 1.5

# conv chunking: output chunks of 118 rows; input chunks of <=128 rows with 5-halo
CHUNK = 118
N_CH = 5  # ceil(512/118)
# per chunk: (input row start, input rows, output row start, output rows)
CH_IN0 = [0, 113, 231, 349, 467]
CH_INN = [123, 128, 128, 128, 45]
CH_OUT0 = [0, 118, 236, 354, 472]
CH_OUTN = [118, 118, 118, 118, 40]

BF16 = mybir.dt.bfloat16
F32 = mybir.dt.float32

SQRT2 = float(np.sqrt(2.0))


def _gauss():
    """Gaussian taps, ULP-adjusted in bf16 so the bf16 window sums to 1.

    Raw bf16 rounding makes the window gain 0.99919, which biases every
    conv output by -0.08% and the final DSSIM by ~5e-3 relative. Nudging
    taps by +/-1 bf16 ULP (greedy, large taps first) recovers sum == 1
    exactly; measured end-to-end error is ~3.5e-4.
    """
    bf = ml_dtypes.bfloat16
    xs = np.arange(WS) - WS // 2
    g = np.exp(-(xs.astype(np.float64) ** 2) / (2.0 * SIGMA ** 2))
    g = (g / g.sum()).astype(np.float32)
    cand = g.astype(bf)
    for _ in range(4):
        for i in np.argsort(-g):
            base = cand.astype(np.float64).sum() - float(cand[i])
            u = np.array(cand[i], dtype=bf).view(np.uint16)
            opts = [
                np.array(u - 1, dtype=np.uint16).view(bf),
                cand[i],
                np.array(u + 1, dtype=np.uint16).view(bf),
            ]
            errs = [abs(base + float(o) - 1.0) for o in opts]
            cand[i] = opts[int(np.argmin(errs))]
    return cand.astype(np.float32)


def _g2(t, g):
    return g[t + 5] if abs(t) <= 5 else 0.0


def _band_mats():
    """Overlap-save band matrices, used by pass1 (as rhs) and, halved, by
    pass2 (as lhsT).

    mid  [128, 118]: M[j, i] = g(j - i - 5)   (input row = out_row - 5 + j)
    first[123, 118]: M[j, i] = g(j - i)       (rows clipped at image top)
    last [ 45,  40]: M[j, i] = g(j - i - 5)
    """
    g = _gauss()
    mid = np.zeros((128, 118), np.float32)
    for j in range(128):
        for i in range(118):
            mid[j, i] = _g2(j - i - 5, g)
    first = np.zeros((123, 118), np.float32)
    for j in range(123):
        for i in range(118):
            first[j, i] = _g2(j - i, g)
    last = np.zeros((45, 40), np.float32)
    for j in range(45):
        for i in range(40):
            last[j, i] = _g2(j - i - 5, g)
    return first, mid, last


def _act_recip(nc, out, in_):
    """activation(func=Reciprocal) without bass's precision guard."""
    eng = nc.scalar
    return eng.add_instruction(
        mybir.InstActivation(
            name=nc.get_next_instruction_name(),
            func=ACTF.Reciprocal,
            ins=[
                eng.lower_ap(in_),
                mybir.ImmediateValue(dtype=mybir.dt.float32, value=0.0),
                mybir.ImmediateValue(dtype=mybir.dt.float32, value=1.0),
                mybir.ImmediateValue(dtype=mybir.dt.float32, value=0.0),
            ],
            outs=[eng.lower_ap(out)],
        )
    )


def build_bass(n_sets=B_LOC * CH):
    nc = bacc.Bacc("TRN2", target_bir_lowering=False, debug=False)

    x_d = nc.dram_tensor("x", [B_LOC, CH, H, W], BF16, kind="ExternalInput")
    y_d = nc.dram_tensor("y", [B_LOC, CH, H, W], BF16, kind="ExternalInput")
    gf_d = nc.dram_tensor("gf", [123, 118], BF16, kind="ExternalInput")
    gm_d = nc.dram_tensor("gm", [128, 118], BF16, kind="ExternalInput")
    gl_d = nc.dram_tensor("gl", [45, 40], BF16, kind="ExternalInput")
    ghf_d = nc.dram_tensor("ghf", [123, 118], BF16, kind="ExternalInput")
    ghm_d = nc.dram_tensor("ghm", [128, 118], BF16, kind="ExternalInput")
    ghl_d = nc.dram_tensor("ghl", [45, 40], BF16, kind="ExternalInput")
    acc_d = nc.dram_tensor("acc", [128, 1], F32, kind="ExternalOutput")

    with tile.TileContext(nc) as tc:
        with (
            tc.tile_pool(name="consts", bufs=1) as consts,
            tc.tile_pool(name="inp", bufs=4) as inp,
            tc.tile_pool(name="prep", bufs=3) as prep,
            tc.tile_pool(name="t1", bufs=3) as t1p,
            tc.tile_pool(name="mapt", bufs=3) as mapt,
            tc.tile_pool(name="p1", bufs=2, space="PSUM") as p1p,
            tc.tile_pool(name="p2", bufs=2, space="PSUM") as p2p,
        ):
            gf = consts.tile([123, 118], BF16, tag="gf", name="gf")
            nc.sync.dma_start(out=gf, in_=gf_d[:, :])
            gm = consts.tile([128, 118], BF16, tag="gm", name="gm")
            nc.sync.dma_start(out=gm, in_=gm_d[:, :])
            gl = consts.tile([45, 40], BF16, tag="gl", name="gl")
            nc.sync.dma_start(out=gl, in_=gl_d[:, :])
            ghf = consts.tile([123, 118], BF16, tag="ghf", name="ghf")
            nc.sync.dma_start(out=ghf, in_=ghf_d[:, :])
            ghm = consts.tile([128, 118], BF16, tag="ghm", name="ghm")
            nc.sync.dma_start(out=ghm, in_=ghm_d[:, :])
            ghl = consts.tile([45, 40], BF16, tag="ghl", name="ghl")
            nc.sync.dma_start(out=ghl, in_=ghl_d[:, :])

            def gpos(c):
                return (gf, gm, gl)[0 if c == 0 else (2 if c == N_CH - 1 else 1)]

            def ghalf(c):
                return (ghf, ghm, ghl)[0 if c == 0 else (2 if c == N_CH - 1 else 1)]

            acc = consts.tile([128, 1], F32, tag="acc", name="acc")
            nc.vector.memset(acc, 0.0)
            rsums = consts.tile([128, 32], F32, tag="rsums", name="rsums")
            nc.vector.memset(rsums, 0.0)
            iround = 0

            for iset in range(n_sets):
                b, c = divmod(iset, CH)
                # ---- load x, y in 5 overlapped row-chunks: [128, 5*512] bf16
                # NOTE: halo rows (beyond each chunk's valid row count) stay
                # garbage on purpose: prep processes them but no consumer
                # (pass1 lhsT, evac, map) ever reads those rows.
                xo = inp.tile([128, N_CH * W], BF16, tag="xo", name="xo")
                yo = inp.tile([128, N_CH * W], BF16, tag="yo", name="yo")
                for k in range(N_CH):
                    r0, nr = CH_IN0[k], CH_INN[k]
                    nc.sync.dma_start(
                        out=xo[0:nr, W * k : W * k + W],
                        in_=x_d[b, c, r0 : r0 + nr, :],
                    )
                    # y goes through the Activation HWDGE queue family so
                    # loads spread over twice the physical DMA queues
                    nc.scalar.dma_start(
                        out=yo[0:nr, W * k : W * k + W],
                        in_=y_d[b, c, r0 : r0 + nr, :],
                    )

                # ---- prep: s, d on vector; s2, d2 as scalar Squares.
                # First set runs in 512-column chunks gated on individual
                # chunk DMAs so the pipeline fills while the load streams.
                st = prep.tile([128, N_CH * W], BF16, tag="s", name="s")
                dt = prep.tile([128, N_CH * W], BF16, tag="d", name="d")
                s2t = prep.tile([128, N_CH * W], BF16, tag="s2", name="s2")
                d2t = prep.tile([128, N_CH * W], BF16, tag="d2", name="d2")
                if iset == 0:
                    for k in range(N_CH):
                        sl = slice(W * k, W * k + W)
                        nc.vector.tensor_add(st[:, sl], xo[:, sl], yo[:, sl])
                        nc.vector.tensor_sub(dt[:, sl], xo[:, sl], yo[:, sl])
                        nc.scalar.activation(
                            out=s2t[:, sl], in_=st[:, sl], func=ACTF.Square
                        )
                        nc.scalar.activation(
                            out=d2t[:, sl], in_=dt[:, sl], func=ACTF.Square
                        )
                else:
                    nc.vector.tensor_add(st, xo, yo)
                    nc.vector.tensor_sub(dt, xo, yo)
                    nc.scalar.activation(out=s2t, in_=st, func=ACTF.Square)
                    nc.scalar.activation(out=d2t, in_=dt, func=ACTF.Square)

                # ---- per 118-row w-chunk
                for m in range(N_CH):
                    w0, pw = CH_IN0[m], CH_INN[m]
                    kin2, p2 = CH_INN[m], CH_OUTN[m]
                    lgh = ghalf(m)

                    # pass1 half 0: [conv_H(s), conv_H(d)] -> ps1a
                    ps1a = p1p.tile([128, 2, W], F32, tag="p1", name="ps1a")
                    for hi, srcm in ((0, st), (1, dt)):
                        for k in range(N_CH):
                            kin = CH_INN[k]
                            o0, on = CH_OUT0[k], CH_OUTN[k]
                            nc.tensor.matmul(
                                ps1a[0:pw, hi, o0 : o0 + on],
                                lhsT=srcm[0:kin, W * k + w0 : W * k + w0 + pw],
                                rhs=gpos(k)[0:kin, 0:on],
                                start=(k == 0),
                                stop=(k == N_CH - 1),
                            )
                    t1sd = t1p.tile([128, 2, W], BF16, tag="t1sd", name="t1sd")
                    nc.scalar.activation(
                        out=t1sd[0:pw, :, :], in_=ps1a[0:pw, :, :],
                        func=ACTF.Copy,
                    )

                    # pass1 half 1: [conv_H(s2), conv_H(d2)] -> ps1b
                    ps1b = p1p.tile([128, 2, W], F32, tag="p1", name="ps1b")
                    for hi, srcm in ((0, s2t), (1, d2t)):
                        for k in range(N_CH):
                            kin = CH_INN[k]
                            o0, on = CH_OUT0[k], CH_OUTN[k]
                            nc.tensor.matmul(
                                ps1b[0:pw, hi, o0 : o0 + on],
                                lhsT=srcm[0:kin, W * k + w0 : W * k + w0 + pw],
                                rhs=gpos(k)[0:kin, 0:on],
                                start=(k == 0),
                                stop=(k == N_CH - 1),
                            )
                    # evacuate plainly, then form the +/- basis in SBUF
                    pq = t1p.tile([128, 2, W], BF16, tag="pq", name="pq")
                    nc.scalar.activation(
                        out=pq[0:pw, :, :], in_=ps1b[0:pw, :, :],
                        func=ACTF.Copy,
                    )
                    t1pm = t1p.tile([128, 2, W], BF16, tag="t1pm", name="t1pm")
                    nc.gpsimd.tensor_sub(
                        t1pm[0:pw, 0, :], pq[0:pw, 0, :], pq[0:pw, 1, :]
                    )
                    nc.gpsimd.tensor_add(
                        t1pm[0:pw, 1, :], pq[0:pw, 0, :], pq[0:pw, 1, :]
                    )

                    # pass2: four matmuls (a matmul output must fit one PSUM
                    # bank, 512 f32), all sharing the same half-band lhsT
                    psA = p2p.tile([118, 2, W], F32, tag="psAB", name="psA")
                    for hi in range(2):
                        nc.tensor.matmul(
                            psA[0:p2, hi, :], lhsT=lgh[0:kin2, 0:p2],
                            rhs=t1sd[0:kin2, hi, :], start=True, stop=True,
                        )
                    psB = p2p.tile([118, 2, W], F32, tag="psAB", name="psB")
                    for hi in range(2):
                        nc.tensor.matmul(
                            psB[0:p2, hi, :], lhsT=lgh[0:kin2, 0:p2],
                            rhs=t1pm[0:kin2, hi, :], start=True, stop=True,
                        )

                    # map stage
                    # ab = (psA * sqrt2)^2 = [S^2/2, D^2/2]
                    ab = mapt.tile([118, 2, W], BF16, tag="ab", name="ab")
                    nc.scalar.activation(
                        out=ab[0:p2, :, :], in_=psA[0:p2, :, :],
                        func=ACTF.Square, scale=SQRT2,
                    )
                    # uv = [u + C1, v + C1]: C1 rides on the ab0 term so it
                    # survives both the difference and the sum
                    uv = mapt.tile([118, 2, W], BF16, tag="uv", name="uv")
                    nc.vector.scalar_tensor_tensor(
                        out=uv[0:p2, 0, :], in0=ab[0:p2, 0, :], scalar=C1,
                        in1=ab[0:p2, 1, :], op0=AOP.add, op1=AOP.subtract,
                    )
                    nc.vector.scalar_tensor_tensor(
                        out=uv[0:p2, 1, :], in0=ab[0:p2, 0, :], scalar=C1,
                        in1=ab[0:p2, 1, :], op0=AOP.add, op1=AOP.add,
                    )
                    # nd = (psB + C1 + C2) - uv = [w-u, h-v]  (uv carries +C1)
                    nd = mapt.tile([118, 2, W], BF16, tag="nd", name="nd")
                    nc.vector.scalar_tensor_tensor(
                        out=nd[0:p2, :, :], in0=psB[0:p2, :, :],
                        scalar=C1 + C2,
                        in1=uv[0:p2, :, :], op0=AOP.add, op1=AOP.subtract,
                    )
                    # num/den = uv * nd (C1 already folded into uv)
                    numden = mapt.tile(
                        [118, 2, W], BF16, tag="numden", name="numden"
                    )
                    nc.vector.tensor_mul(
                        numden[0:p2, :, :], uv[0:p2, :, :], nd[0:p2, :, :]
                    )
                    rb = mapt.tile([118, W], BF16, tag="rb", name="rb")
                    _act_recip(nc, rb[0:p2, :], numden[0:p2, 1, :])
                    scr = mapt.tile([118, W], BF16, tag="scr", name="scr")
                    nc.vector.scalar_tensor_tensor(
                        out=scr[0:p2, :], in0=numden[0:p2, 0, :], scalar=1.0,
                        in1=rb[0:p2, :], op0=AOP.mult, op1=AOP.mult,
                        accum_out=rsums[0:p2, iround : iround + 1],
                    )
                    iround += 1

            nc.vector.tensor_reduce(
                out=acc, in_=rsums, op=AOP.add, axis=mybir.AxisListType.X
            )
            nc.scalar.dma_start(out=acc_d[:, :], in_=acc)

    nc.finalize()
    return nc


_NC_CACHE = None


def _get_nc():
    global _NC_CACHE
    if _NC_CACHE is None:
        _NC_CACHE = build_bass()
    return _NC_CACHE


def make_in_maps(x: np.ndarray, y: np.ndarray):
    """Shard full f32 inputs into per-core bf16 input maps."""
    bf = ml_dtypes.bfloat16
    x = np.asarray(x).astype(bf)
    y = np.asarray(y).astype(bf)

    first, mid, last = _band_mats()
    consts = {
        "gf": first.astype(bf),
        "gm": mid.astype(bf),
        "gl": last.astype(bf),
        # exact halves (pure exponent shift in bf16)
        "ghf": (first.astype(bf).astype(np.float32) / 2).astype(bf),
        "ghm": (mid.astype(bf).astype(np.float32) / 2).astype(bf),
        "ghl": (last.astype(bf).astype(np.float32) / 2).astype(bf),
    }

    in_maps = []
    for core in range(N_CORES):
        b0 = core * B_LOC
        in_maps.append(
            {
                "x": np.ascontiguousarray(x[b0 : b0 + B_LOC]),
                "y": np.ascontiguousarray(y[b0 : b0 + B_LOC]),
                **consts,
            }
        )
    return in_maps


def kernel(x: np.ndarray, y: np.ndarray) -> np.ndarray:
    nc = _get_nc()
    in_maps = make_in_maps(x, y)
    res = run_bass_kernel_spmd(nc, in_maps, core_ids=list(range(N_CORES)))
    total = np.float64(0.0)
    for r in res.results:
        total += np.asarray(r["acc"], dtype=np.float64).sum()
    n_pix = FULL_B * CH * H * W
    return np.float32(1.0 - total / n_pix)


if __name__ == "__main__":
    rng = np.random.default_rng(0)
    x = rng.random((FULL_B, CH, H, W), dtype=np.float32)
    y = rng.random((FULL_B, CH, H, W), dtype=np.float32)
    print("kernel:", kernel(x, y))
